# revision 2
# baseline (speedup 1.0000x reference)
"""Trainium2 Bass kernel for nn_AMPBlock0 (BigVGAN AMP block):
x -> SnakeBeta-Activation1d -> Conv1d(512,512,k3) -> SnakeBeta-Activation1d
  -> Conv1d(512,512,k3) -> + x

Data-parallel over batch B=8 across 8 NeuronCores (one sample per core,
zero collectives). ~394us HW exec, rel err 5.2e-3.

Conv notes: bf16 12-tap convs are kept deliberately. fp8-e4m3 DoubleRow
(with hi/lo splits for precision) was implemented and measured LOAD-BOUND
on TRN2: ldweights streams at 1.2 GHz/col while DR doubles the loaded
columns (256) and halves compute (255 cy at FD=510), so each DR pass
costs ~215ns vs ~212ns for a bf16 tap that hides its 128-col load --
every fp8 config (plain/act-split/weight-split/both, with or without
cross-segment weight reuse) arithmetically loses to bf16. int8 matmul
is not in TRN2's legal matmult dtypes (Cayman ISA). PSUM matmul outputs
are limited to one 2KB bank (<=510 fp32 cols per segment).

Activation-chain changes vs the v2 baseline (DVE 356us -> ~343us,
ACT 327us -> ~243us measured):
 - activation range reduction via hh(f16,TS-4x) -> kk(int16 round, TS-4x)
   -> frac2(TT-2x) -> Sin(2pi*frac2), replacing the kk/mm pair whose fp32
   STT ran at 1x; the quarter-period phase shift is folded into the z
   tiles (host adds delta=(3/16)P per channel; conv bias folds undo it).
 - sv computed as TT(ez6, z) at 2x instead of a 1x STT; for act1 the /6
   is free via a host-precomputed x/6 tensor (ez6/dz6 read x6 directly).
 - bf16 output (host upcasts to fp32): halves output DMA.
"""
import sys
if '/opt/trn_rl_repo' not in sys.path:
    sys.path.insert(0, '/opt/trn_rl_repo')

import numpy as np
import ml_dtypes

import concourse.bass as bass
import concourse.mybir as mybir
import concourse.tile as tile
from concourse import bacc
from concourse.bass_utils import run_bass_kernel_spmd

F32 = mybir.dt.float32
BF16 = mybir.dt.bfloat16
F16 = mybir.dt.float16
I16 = mybir.dt.int16
AOP = mybir.AluOpType
AF = mybir.ActivationFunctionType

C = 512
T = 8192
NCC = 4
P = 128
PAD = 4
EPS = 1e-9
TWOPI = 2.0 * np.pi

_NC_CACHE = {}

import os
GG_ENGINE = os.environ.get('GG_ENGINE', 'dve')   # 'dve' | 'act'

CHUNKS = []
_o = 0
for _s in [252, 506, 760] + [1016] * 6 + [322, 256]:
    CHUNKS.append((_o, _s))
    _o += _s
assert _o == T


def _segments(width, cap=510):
    segs = []
    c = 0
    while c < width:
        w = min(cap, width - c)
        segs.append((c, w))
        c += w
    return segs


def build_nc():
    nc = bacc.Bacc(None, num_swdge_queues=4)
    xpb = nc.declare_dram_parameter("xpb", [C, T + 2 * PAD], BF16,
                                    isOutput=False)
    x6b = nc.declare_dram_parameter("x6b", [C, T + 2 * PAD], BF16,
                                    isOutput=False)
    NW = 2 * 3 * NCC * NCC
    # bf16 weights, trailing identity tile (residual tap)
    wts = nc.declare_dram_parameter("wts", [P, (NW + 1) * P], BF16,
                                    isOutput=False)
    prm = nc.declare_dram_parameter("prm", [P, NCC * 16], F32, isOutput=False)
    outd = nc.declare_dram_parameter("out", [C, T], BF16, isOutput=True)

    with tile.TileContext(nc) as tc:
        with tc.tile_pool(name="wp", bufs=1) as wp, \
             tc.tile_pool(name="pp", bufs=1) as ppool, \
             tc.tile_pool(name="xbp", bufs=8) as xbp, \
             tc.tile_pool(name="tp", bufs=4) as tp, \
             tc.tile_pool(name="zp", bufs=8) as zp, \
             tc.tile_pool(name="iop", bufs=4) as iop, \
             tc.tile_pool(name="ps", bufs=4, space="PSUM") as psp:

            w_all = wp.tile([P, (NW + 1) * P], BF16)

            def wview(conv, dk, ci, co):
                idx = ((conv * 3 + dk) * NCC + ci) * NCC + co
                return w_all[:, idx * P:(idx + 1) * P]

            w_ident = w_all[:, NW * P:(NW + 1) * P]

            prmt = ppool.tile([P, NCC * 16], F32, tag="prm")
            nc.sync.dma_start(out=prmt[:], in_=prm[:])
            pb = ppool.tile([P, 1], F32, tag="pb")
            nc.vector.memset(pb[:], float(np.pi / 2))

            def pcol(cc, j):
                return prmt[:, cc * 16 + j:cc * 16 + j + 1]

            last_i = len(CHUNKS) - 1

            def load_x(ci_chunk):
                o0, S = CHUNKS[ci_chunk]
                Ex = S + 8
                tiles = []
                for cc in range(NCC):
                    xb = xbp.tile([P, Ex], BF16, tag="xb", bufs=12)
                    nc.sync.dma_start(out=xb[:],
                                      in_=xpb[cc * P:(cc + 1) * P, o0:o0 + Ex])
                    x6 = xbp.tile([P, Ex], BF16, tag="x6", bufs=12)
                    nc.sync.dma_start(out=x6[:],
                                      in_=x6b[cc * P:(cc + 1) * P, o0:o0 + Ex])
                    tiles.append((xb, x6))
                return tiles

            def act_chain(w, act, ez6s, dzs, zbases, dst_tag):
                """Shared tail of both act stages; returns dst (z) tiles."""
                svs, fracs, cBs, cAs, scs, ggs, dsts = [], [], [], [], [], [], []
                for cc in range(NCC):
                    cB = tp.tile([P, w], BF16, tag="cB", bufs=4)
                    nc.scalar.activation(cB[:], dzs[cc][:], AF.Sin,
                                         bias=pb[:],
                                         scale=pcol(cc, 6 * act + 1))
                    cBs.append(cB)
                for cc in range(NCC):
                    sv = tp.tile([P, w], BF16, tag="sv", bufs=5)
                    nc.vector.tensor_add(sv[:], ez6s[cc][:], zbases[cc])
                    svs.append(sv)
                for cc in range(NCC):
                    hh = tp.tile([P, w], F16, tag="hh", bufs=2)
                    nc.vector.tensor_scalar(hh[:], svs[cc][:],
                                            pcol(cc, 6 * act + 0), None,
                                            AOP.mult)
                    kk = tp.tile([P, w], I16, tag="kk", bufs=2)
                    nc.vector.tensor_scalar(kk[:], hh[:], 1.0, None,
                                            AOP.mult)
                    frac = tp.tile([P, w], F16, tag="frac", bufs=4)
                    nc.vector.tensor_tensor(frac[:], hh[:], kk[:],
                                            AOP.subtract)
                    fracs.append(frac)
                for cc in range(NCC):
                    cA = tp.tile([P, w], BF16, tag="cA", bufs=4)
                    nc.scalar.activation(cA[:], fracs[cc][:], AF.Sin,
                                         bias=0.0, scale=float(TWOPI))
                    cAs.append(cA)
                for cc in range(NCC):
                    sc = tp.tile([P, w], BF16, tag="sc", bufs=4)
                    nc.vector.tensor_mul(sc[:], cAs[cc][:], cBs[cc][:])
                    scs.append(sc)
                for cc in range(NCC):
                    gg = tp.tile([P, w], BF16, tag="gg", bufs=4)
                    if GG_ENGINE == 'dve':
                        nc.vector.tensor_scalar(gg[:], scs[cc][:],
                                                pcol(cc, 6 * act + 2), None,
                                                AOP.mult)
                    else:
                        nc.scalar.activation(gg[:], scs[cc][:], AF.Copy,
                                             scale=pcol(cc, 6 * act + 2))
                    ggs.append(gg)
                for cc in range(NCC):
                    dst = zp.tile([P, w], BF16, tag=dst_tag, bufs=6)
                    nc.vector.tensor_add(dst[:], svs[cc][:], ggs[cc][:])
                    dsts.append(dst)
                return dsts

            def emit_act1(ci, xts):
                o0c, Sc = CHUNKS[ci]
                w = Sc + 6
                ez6s, dzs, zb = [], [], []
                for cc in range(NCC):
                    x6t = xts[cc][1]
                    dz = tp.tile([P, w], BF16, tag="dz", bufs=4)
                    nc.vector.tensor_tensor(dz[:], x6t[:, 0:w],
                                            x6t[:, 2:w + 2], AOP.subtract)
                    dzs.append(dz)
                    zb.append(xts[cc][0][:, 1:w + 1])
                for cc in range(NCC):
                    x6t = xts[cc][1]
                    ez6 = tp.tile([P, w], BF16, tag="ez6", bufs=4)
                    nc.vector.tensor_add(ez6[:], x6t[:, 0:w], x6t[:, 2:w + 2])
                    ez6s.append(ez6)
                z1 = act_chain(w, 0, ez6s, dzs, zb, "z1")
                for cc in range(NCC):
                    if ci == 0:
                        nc.scalar.activation(z1[cc][:, 2:3], pcol(cc, 3),
                                             AF.Copy)
                    if ci == last_i:
                        nc.scalar.activation(z1[cc][:, Sc + 3:Sc + 4],
                                             pcol(cc, 3), AF.Copy)
                return z1

            def emit_conv1(ci_chunk, z1s):
                o0, S = CHUNKS[ci_chunk]
                first = ci_chunk == 0
                last = ci_chunk == last_i
                E2 = S + 4
                z2s = []
                for _cc in range(NCC):
                    z2t = zp.tile([P, E2], BF16, tag="z2", bufs=8)
                    z2s.append(z2t)
                segs1 = _segments(E2, 510)
                for co in range(NCC):
                    pss = []
                    for (c0, w) in segs1:
                        pst = psp.tile([P, w], F32, tag="cp1", bufs=4)
                        pss.append(pst)
                    n = 0
                    for ci in range(NCC):
                        for dk in range(3):
                            for si, (c0, w) in enumerate(segs1):
                                nc.tensor.matmul(
                                    pss[si][:, :w],
                                    wview(0, dk, ci, co),
                                    z1s[ci][:, c0 + dk:c0 + dk + w],
                                    start=(n == 0), stop=(n == 11))
                            n += 1
                    for si, (c0, w) in enumerate(segs1):
                        nc.scalar.activation(z2s[co][:, c0:c0 + w],
                                             pss[si][:, :w],
                                             AF.Identity, bias=pcol(co, 5),
                                             scale=1.0)
                for cc in range(NCC):
                    z2 = z2s[cc]
                    if first:
                        nc.scalar.activation(z2[:, 0:1], z2[:, 2:3], AF.Copy)
                        nc.scalar.activation(z2[:, 1:2], z2[:, 2:3], AF.Copy)
                    if last:
                        nc.scalar.activation(z2[:, S + 2:S + 3],
                                             z2[:, S + 1:S + 2], AF.Copy)
                        nc.scalar.activation(z2[:, S + 3:S + 4],
                                             z2[:, S + 1:S + 2], AF.Copy)
                return z2s

            def emit_act2(ci_chunk, z2s):
                o0, S = CHUNKS[ci_chunk]
                first = ci_chunk == 0
                last = ci_chunk == last_i
                w = S + 2
                ez6s, dzs, zb = [], [], []
                for cc in range(NCC):
                    z2 = z2s[cc]
                    dz = tp.tile([P, w], BF16, tag="dz", bufs=4)
                    nc.vector.tensor_tensor(dz[:], z2[:, 0:w], z2[:, 2:w + 2],
                                            AOP.subtract)
                    dzs.append(dz)
                    zb.append(z2[:, 1:w + 1])
                for cc in range(NCC):
                    z2 = z2s[cc]
                    ez = tp.tile([P, w], BF16, tag="ez", bufs=4)
                    nc.vector.tensor_add(ez[:], z2[:, 0:w], z2[:, 2:w + 2])
                    ez6 = tp.tile([P, w], BF16, tag="ez6", bufs=4)
                    nc.vector.tensor_scalar(ez6[:], ez[:], 1.0 / 6.0, None,
                                            AOP.mult)
                    ez6s.append(ez6)
                z3 = act_chain(w, 1, ez6s, dzs, zb, "z3")
                for cc in range(NCC):
                    if first:
                        nc.scalar.activation(z3[cc][:, 0:1], pcol(cc, 9),
                                             AF.Copy)
                    if last:
                        nc.scalar.activation(z3[cc][:, S + 1:S + 2],
                                             pcol(cc, 9), AF.Copy)
                return z3

            def emit_conv2(ci_chunk, z3s, xts, evict_on_dve=False):
                o0, S = CHUNKS[ci_chunk]
                segs2 = _segments(S, 508)
                for pair in ((0, 1), (2, 3)):
                    pssm = {}
                    for co in pair:
                        pssm[co] = []
                        for si, (c0, w) in enumerate(segs2):
                            pst = psp.tile([P, w], F32, tag="cp2", bufs=4)
                            pssm[co].append(pst)
                    n = 0
                    for ci in range(NCC):
                        for dk in range(3):
                            for co in pair:
                                for si, (c0, w) in enumerate(segs2):
                                    nc.tensor.matmul(
                                        pssm[co][si][:, :w],
                                        wview(1, dk, ci, co),
                                        z3s[ci][:, c0 + dk:c0 + dk + w],
                                        start=(n == 0), stop=False)
                            n += 1
                    for co in pair:
                        for si, (c0, w) in enumerate(segs2):
                            nc.tensor.matmul(
                                pssm[co][si][:, :w],
                                w_ident,
                                xts[co][0][:, c0 + 4:c0 + 4 + w],
                                start=False, stop=True)
                    for co in pair:
                        for si, (c0, w) in enumerate(segs2):
                            of = iop.tile([P, w], BF16, tag="of")
                            if evict_on_dve:
                                nc.vector.tensor_scalar(
                                    of[:], pssm[co][si][:, :w],
                                    pcol(co, 11), None, AOP.add)
                            else:
                                nc.scalar.activation(
                                    of[:], pssm[co][si][:, :w], AF.Identity,
                                    bias=pcol(co, 11), scale=1.0)
                            nc.sync.dma_start(
                                out=outd[co * P:(co + 1) * P,
                                         o0 + c0:o0 + c0 + w],
                                in_=of[:])

            # ---- prologue ----
            x_tiles = {0: load_x(0), 1: load_x(1)}
            WQ = (NW + 1) * P // 4
            for q in range(4):
                nc.sync.dma_start(out=w_all[:, q * WQ:(q + 1) * WQ],
                                  in_=wts[:, q * WQ:(q + 1) * WQ])
            z1 = emit_act1(0, x_tiles[0])
            z2cur = emit_conv1(0, z1)

            # ---- software pipeline ----
            for c in range(len(CHUNKS)):
                if c + 2 <= last_i and (c + 2) not in x_tiles:
                    x_tiles[c + 2] = load_x(c + 2)
                z1n = (emit_act1(c + 1, x_tiles[c + 1])
                       if c + 1 <= last_i else None)
                z3 = emit_act2(c, z2cur)
                if z1n is not None:
                    z2cur = emit_conv1(c + 1, z1n)
                emit_conv2(c, z3, x_tiles.pop(c),
                           evict_on_dve=(c >= last_i - 1))
    nc.compile()
    return nc


def _host_prep(x, v1, g1, bias1, v2, g2, bias2, alpha1, beta1, alpha2, beta2):
    f32 = np.float32

    def wn(v, g):
        nrm = np.sqrt((v * v).sum(axis=(1, 2), keepdims=True))
        return (g[:, None, None] * v / nrm).astype(f32)

    def bf(a):
        return np.asarray(a, dtype=f32).astype(ml_dtypes.bfloat16)

    a1 = np.exp(alpha1).astype(f32)
    a2 = np.exp(alpha2).astype(f32)
    rbp1 = ((4.0 / 3.0) / (2.0 * np.exp(beta1) + EPS)).astype(f32)
    rbp2 = ((4.0 / 3.0) / (2.0 * np.exp(beta2) + EPS)).astype(f32)
    P1 = (TWOPI / (1.5 * a1)).astype(f32)
    P2 = (TWOPI / (1.5 * a2)).astype(f32)
    d1 = ((3.0 / 16.0) * P1).astype(f32)
    d2 = ((3.0 / 16.0) * P2).astype(f32)

    W1 = wn(v1, g1) * f32(0.75)
    W2 = wn(v2, g2) * f32(0.75)

    NW = 2 * 3 * NCC * NCC
    wflat = np.zeros((P, (NW + 1) * P), dtype=ml_dtypes.bfloat16)
    wflat[:, NW * P:] = np.eye(P, dtype=f32).astype(ml_dtypes.bfloat16)
    Wq = {}
    for conv, W in ((0, W1), (1, W2)):
        Wb = bf(W)
        Wq[conv] = Wb.astype(f32)
        for dk in range(3):
            for ci in range(NCC):
                for co in range(NCC):
                    idx = ((conv * 3 + dk) * NCC + ci) * NCC + co
                    blk = Wb[co * P:(co + 1) * P, ci * P:(ci + 1) * P, dk]
                    wflat[:, idx * P:(idx + 1) * P] = blk.T

    sent1 = (P1 / 4 - rbp1).astype(f32)
    sent2 = (P2 / 4 - rbp2).astype(f32)

    beff1 = (bias1 + np.einsum('oik,i->o', Wq[0], rbp1 - P1 / 4) + d2
             ).astype(f32)
    beff2 = (bias2 + np.einsum('oik,i->o', Wq[1], rbp2 - P2 / 4) - d1
             ).astype(f32)

    prm_c = np.zeros((C, 16), dtype=f32)
    prm_c[:, 0] = 1.5 * a1 / TWOPI   # hh scale act1
    prm_c[:, 1] = 1.5 * a1           # cB scale on dz6
    prm_c[:, 2] = -rbp1              # gg scale (cA = +cos)
    prm_c[:, 3] = sent1
    prm_c[:, 5] = beff1
    prm_c[:, 6] = 1.5 * a2 / TWOPI
    prm_c[:, 7] = 0.25 * a2          # cB scale on raw dz
    prm_c[:, 8] = -rbp2
    prm_c[:, 9] = sent2
    prm_c[:, 11] = beff2
    prm = np.ascontiguousarray(
        prm_c.reshape(NCC, P, 16).transpose(1, 0, 2).reshape(P, NCC * 16))

    xpad = np.pad(x + d1[None, :, None], ((0, 0), (0, 0), (PAD, PAD)),
                  mode='edge').astype(f32)
    xpb = xpad.astype(ml_dtypes.bfloat16)
    x6b = (xpad / 6.0).astype(ml_dtypes.bfloat16)
    return xpb, x6b, wflat, prm


def kernel(x, v1, g1, bias1, v2, g2, bias2, alpha1, beta1, alpha2, beta2,
           _profile=False):
    x = np.ascontiguousarray(x, dtype=np.float32)
    xpb, x6b, wflat, prm = _host_prep(
        x, v1, g1, bias1, v2, g2, bias2, alpha1, beta1, alpha2, beta2)
    if 'nc' not in _NC_CACHE:
        _NC_CACHE['nc'] = build_nc()
    nc = _NC_CACHE['nc']
    B = x.shape[0]
    assert B == 8, f"expected B=8, got {B}"
    in_maps = [{"xpb": np.ascontiguousarray(xpb[b]),
                "x6b": np.ascontiguousarray(x6b[b]),
                "wts": wflat, "prm": prm} for b in range(B)]
    last_exc = None
    for attempt in range(3):
        try:
            res = run_bass_kernel_spmd(nc, in_maps, list(range(8)),
                                       trace=_profile)
            break
        except Exception as e:
            last_exc = e
            import time
            time.sleep(2.0)
    else:
        raise last_exc
    out = np.stack([res.results[b]["out"].astype(np.float32)
                    for b in range(B)])
    if _profile:
        kernel.last_results = res
    return out


# revision 3
# speedup vs baseline: 1.0107x; 1.0107x over previous
"""Trainium2 Bass kernel for nn_AMPBlock0 (BigVGAN AMP block):
x -> SnakeBeta-Activation1d -> Conv1d(512,512,k3) -> SnakeBeta-Activation1d
  -> Conv1d(512,512,k3) -> + x

Data-parallel over batch B=8 across 8 NeuronCores (one sample per core,
zero collectives). ~393-397us HW exec, rel err 5.2e-3. PE-bound: 356us
tensor-engine active (89%), with fill 18us / drain 12us / gaps 10us.
Activation chains lead their consuming convs by a full pipeline
iteration; weights are packed in consumption order so conv1-co0's 12
tiles arrive in the first small DMA. fp8 DoubleRow and int8 conv paths
were measured/ruled out (DR is ldweights-bound at 1.2GHz/col on TRN2;
int8 matmul is not in the Cayman ISA).

Changes over the 405us v2 baseline (bf16 convs kept: fp8 DoubleRow measured
load-bound on TRN2 -- ldweights runs at 1.2 GHz/col and DR doubles the
loaded columns while halving compute, so every DR config loses):
 - activation range reduction via hh(f16,TS-4x) -> kk(int16 round, TS-4x)
   -> frac2(TT-2x) -> Sin(2pi*frac2), replacing the kk/mm pair whose fp32
   STT ran at 1x; the quarter-period phase shift is folded into the z
   tiles (host adds delta=(3/16)P per channel; conv bias folds undo it).
 - sv computed as TT(ez6, z) at 2x instead of a 1x STT; for act1 the /6
   is free via a host-precomputed x/6 tensor (ez6/dz6 read x6 directly).
 - bf16 output (host upcasts to fp32): halves output DMA.
"""
import sys
if '/opt/trn_rl_repo' not in sys.path:
    sys.path.insert(0, '/opt/trn_rl_repo')

import numpy as np
import ml_dtypes

import concourse.bass as bass
import concourse.mybir as mybir
import concourse.tile as tile
from concourse import bacc
from concourse.bass_utils import run_bass_kernel_spmd

F32 = mybir.dt.float32
BF16 = mybir.dt.bfloat16
F16 = mybir.dt.float16
I16 = mybir.dt.int16
AOP = mybir.AluOpType
AF = mybir.ActivationFunctionType

C = 512
T = 8192
NCC = 4
P = 128
PAD = 4
EPS = 1e-9
TWOPI = 2.0 * np.pi

_NC_CACHE = {}

import os
GG_ENGINE = os.environ.get('GG_ENGINE', 'dve')   # 'dve' | 'act'

CHUNKS = []
_o = 0
for _s in [252, 506, 760] + [1016] * 6 + [322, 256]:
    CHUNKS.append((_o, _s))
    _o += _s
assert _o == T


def _segments(width, cap=510):
    segs = []
    c = 0
    while c < width:
        w = min(cap, width - c)
        segs.append((c, w))
        c += w
    return segs


def build_nc():
    nc = bacc.Bacc(None, num_swdge_queues=4)
    xpb = nc.declare_dram_parameter("xpb", [C, T + 2 * PAD], BF16,
                                    isOutput=False)
    x6b = nc.declare_dram_parameter("x6b", [C, T + 2 * PAD], BF16,
                                    isOutput=False)
    NW = 2 * 3 * NCC * NCC
    # bf16 weights, trailing identity tile (residual tap)
    wts = nc.declare_dram_parameter("wts", [P, (NW + 1) * P], BF16,
                                    isOutput=False)
    prm = nc.declare_dram_parameter("prm", [P, NCC * 16], F32, isOutput=False)
    outd = nc.declare_dram_parameter("out", [C, T], BF16, isOutput=True)

    with tile.TileContext(nc) as tc:
        with tc.tile_pool(name="wp", bufs=1) as wp, \
             tc.tile_pool(name="pp", bufs=1) as ppool, \
             tc.tile_pool(name="xbp", bufs=8) as xbp, \
             tc.tile_pool(name="tp", bufs=4) as tp, \
             tc.tile_pool(name="zp", bufs=8) as zp, \
             tc.tile_pool(name="iop", bufs=4) as iop, \
             tc.tile_pool(name="ps", bufs=4, space="PSUM") as psp:

            w_all = wp.tile([P, (NW + 1) * P], BF16)

            def wview(conv, dk, ci, co):
                idx = conv * 48 + co * 12 + dk * NCC + ci
                return w_all[:, idx * P:(idx + 1) * P]

            w_ident = w_all[:, NW * P:(NW + 1) * P]

            prmt = ppool.tile([P, NCC * 16], F32, tag="prm")
            nc.sync.dma_start(out=prmt[:], in_=prm[:])
            pb = ppool.tile([P, 1], F32, tag="pb")
            nc.vector.memset(pb[:], float(np.pi / 2))

            def pcol(cc, j):
                return prmt[:, cc * 16 + j:cc * 16 + j + 1]

            last_i = len(CHUNKS) - 1

            def load_x(ci_chunk):
                o0, S = CHUNKS[ci_chunk]
                Ex = S + 8
                tiles = []
                for cc in range(NCC):
                    xb = xbp.tile([P, Ex], BF16, tag="xb", bufs=12)
                    nc.sync.dma_start(out=xb[:],
                                      in_=xpb[cc * P:(cc + 1) * P, o0:o0 + Ex])
                    x6 = xbp.tile([P, Ex], BF16, tag="x6", bufs=8)
                    nc.sync.dma_start(out=x6[:],
                                      in_=x6b[cc * P:(cc + 1) * P, o0:o0 + Ex])
                    tiles.append((xb, x6))
                return tiles

            def act_chain(w, act, ez6s, dzs, zbases, dst_tag):
                """Shared tail of both act stages; returns dst (z) tiles."""
                svs, fracs, cBs, cAs, scs, ggs, dsts = [], [], [], [], [], [], []
                for cc in range(NCC):
                    cB = tp.tile([P, w], BF16, tag="cB", bufs=4)
                    nc.scalar.activation(cB[:], dzs[cc][:], AF.Sin,
                                         bias=pb[:],
                                         scale=pcol(cc, 6 * act + 1))
                    cBs.append(cB)
                for cc in range(NCC):
                    sv = tp.tile([P, w], BF16, tag="sv", bufs=5)
                    nc.vector.tensor_add(sv[:], ez6s[cc][:], zbases[cc])
                    svs.append(sv)
                for cc in range(NCC):
                    hh = tp.tile([P, w], F16, tag="hh", bufs=2)
                    nc.vector.tensor_scalar(hh[:], svs[cc][:],
                                            pcol(cc, 6 * act + 0), None,
                                            AOP.mult)
                    kk = tp.tile([P, w], I16, tag="kk", bufs=2)
                    nc.vector.tensor_scalar(kk[:], hh[:], 1.0, None,
                                            AOP.mult)
                    frac = tp.tile([P, w], F16, tag="frac", bufs=4)
                    nc.vector.tensor_tensor(frac[:], hh[:], kk[:],
                                            AOP.subtract)
                    fracs.append(frac)
                for cc in range(NCC):
                    cA = tp.tile([P, w], BF16, tag="cA", bufs=4)
                    nc.scalar.activation(cA[:], fracs[cc][:], AF.Sin,
                                         bias=0.0, scale=float(TWOPI))
                    cAs.append(cA)
                for cc in range(NCC):
                    sc = tp.tile([P, w], BF16, tag="sc", bufs=4)
                    nc.vector.tensor_mul(sc[:], cAs[cc][:], cBs[cc][:])
                    scs.append(sc)
                for cc in range(NCC):
                    gg = tp.tile([P, w], BF16, tag="gg", bufs=4)
                    if GG_ENGINE == 'dve':
                        nc.vector.tensor_scalar(gg[:], scs[cc][:],
                                                pcol(cc, 6 * act + 2), None,
                                                AOP.mult)
                    else:
                        nc.scalar.activation(gg[:], scs[cc][:], AF.Copy,
                                             scale=pcol(cc, 6 * act + 2))
                    ggs.append(gg)
                for cc in range(NCC):
                    dst = zp.tile([P, w], BF16, tag=dst_tag,
                                  bufs=10 if dst_tag == "z1" else 6)
                    nc.vector.tensor_add(dst[:], svs[cc][:], ggs[cc][:])
                    dsts.append(dst)
                return dsts

            def emit_act1(ci, xts):
                o0c, Sc = CHUNKS[ci]
                w = Sc + 6
                ez6s, dzs, zb = [], [], []
                for cc in range(NCC):
                    x6t = xts[cc][1]
                    dz = tp.tile([P, w], BF16, tag="dz", bufs=4)
                    nc.vector.tensor_tensor(dz[:], x6t[:, 0:w],
                                            x6t[:, 2:w + 2], AOP.subtract)
                    dzs.append(dz)
                    zb.append(xts[cc][0][:, 1:w + 1])
                for cc in range(NCC):
                    x6t = xts[cc][1]
                    ez6 = tp.tile([P, w], BF16, tag="ez6", bufs=4)
                    nc.vector.tensor_add(ez6[:], x6t[:, 0:w], x6t[:, 2:w + 2])
                    ez6s.append(ez6)
                z1 = act_chain(w, 0, ez6s, dzs, zb, "z1")
                for cc in range(NCC):
                    if ci == 0:
                        nc.scalar.activation(z1[cc][:, 2:3], pcol(cc, 3),
                                             AF.Copy)
                    if ci == last_i:
                        nc.scalar.activation(z1[cc][:, Sc + 3:Sc + 4],
                                             pcol(cc, 3), AF.Copy)
                return z1

            def emit_conv1(ci_chunk, z1s):
                o0, S = CHUNKS[ci_chunk]
                first = ci_chunk == 0
                last = ci_chunk == last_i
                E2 = S + 4
                z2s = []
                for _cc in range(NCC):
                    z2t = zp.tile([P, E2], BF16, tag="z2", bufs=8)
                    z2s.append(z2t)
                segs1 = _segments(E2, 510)
                for co in range(NCC):
                    pss = []
                    for (c0, w) in segs1:
                        pst = psp.tile([P, w], F32, tag="cp1", bufs=4)
                        pss.append(pst)
                    n = 0
                    for ci in range(NCC):
                        for dk in range(3):
                            for si, (c0, w) in enumerate(segs1):
                                nc.tensor.matmul(
                                    pss[si][:, :w],
                                    wview(0, dk, ci, co),
                                    z1s[ci][:, c0 + dk:c0 + dk + w],
                                    start=(n == 0), stop=(n == 11))
                            n += 1
                    for si, (c0, w) in enumerate(segs1):
                        nc.scalar.activation(z2s[co][:, c0:c0 + w],
                                             pss[si][:, :w],
                                             AF.Identity, bias=pcol(co, 5),
                                             scale=1.0)
                for cc in range(NCC):
                    z2 = z2s[cc]
                    if first:
                        nc.scalar.activation(z2[:, 0:1], z2[:, 2:3], AF.Copy)
                        nc.scalar.activation(z2[:, 1:2], z2[:, 2:3], AF.Copy)
                    if last:
                        nc.scalar.activation(z2[:, S + 2:S + 3],
                                             z2[:, S + 1:S + 2], AF.Copy)
                        nc.scalar.activation(z2[:, S + 3:S + 4],
                                             z2[:, S + 1:S + 2], AF.Copy)
                return z2s

            def emit_act2(ci_chunk, z2s):
                o0, S = CHUNKS[ci_chunk]
                first = ci_chunk == 0
                last = ci_chunk == last_i
                w = S + 2
                ez6s, dzs, zb = [], [], []
                for cc in range(NCC):
                    z2 = z2s[cc]
                    dz = tp.tile([P, w], BF16, tag="dz", bufs=4)
                    nc.vector.tensor_tensor(dz[:], z2[:, 0:w], z2[:, 2:w + 2],
                                            AOP.subtract)
                    dzs.append(dz)
                    zb.append(z2[:, 1:w + 1])
                for cc in range(NCC):
                    z2 = z2s[cc]
                    ez = tp.tile([P, w], BF16, tag="ez", bufs=4)
                    nc.vector.tensor_add(ez[:], z2[:, 0:w], z2[:, 2:w + 2])
                    ez6 = tp.tile([P, w], BF16, tag="ez6", bufs=4)
                    nc.vector.tensor_scalar(ez6[:], ez[:], 1.0 / 6.0, None,
                                            AOP.mult)
                    ez6s.append(ez6)
                z3 = act_chain(w, 1, ez6s, dzs, zb, "z3")
                for cc in range(NCC):
                    if first:
                        nc.scalar.activation(z3[cc][:, 0:1], pcol(cc, 9),
                                             AF.Copy)
                    if last:
                        nc.scalar.activation(z3[cc][:, S + 1:S + 2],
                                             pcol(cc, 9), AF.Copy)
                return z3

            def emit_conv2(ci_chunk, z3s, xts, evict_on_dve=False):
                o0, S = CHUNKS[ci_chunk]
                segs2 = _segments(S, 508)
                for pair in ((0, 1), (2, 3)):
                    pssm = {}
                    for co in pair:
                        pssm[co] = []
                        for si, (c0, w) in enumerate(segs2):
                            pst = psp.tile([P, w], F32, tag="cp2", bufs=4)
                            pssm[co].append(pst)
                    n = 0
                    for ci in range(NCC):
                        for dk in range(3):
                            for co in pair:
                                for si, (c0, w) in enumerate(segs2):
                                    nc.tensor.matmul(
                                        pssm[co][si][:, :w],
                                        wview(1, dk, ci, co),
                                        z3s[ci][:, c0 + dk:c0 + dk + w],
                                        start=(n == 0), stop=False)
                            n += 1
                    for co in pair:
                        for si, (c0, w) in enumerate(segs2):
                            nc.tensor.matmul(
                                pssm[co][si][:, :w],
                                w_ident,
                                xts[co][0][:, c0 + 4:c0 + 4 + w],
                                start=False, stop=True)
                    for co in pair:
                        for si, (c0, w) in enumerate(segs2):
                            of = iop.tile([P, w], BF16, tag="of")
                            if evict_on_dve:
                                nc.vector.tensor_scalar(
                                    of[:], pssm[co][si][:, :w],
                                    pcol(co, 11), None, AOP.add)
                            else:
                                nc.scalar.activation(
                                    of[:], pssm[co][si][:, :w], AF.Identity,
                                    bias=pcol(co, 11), scale=1.0)
                            nc.sync.dma_start(
                                out=outd[co * P:(co + 1) * P,
                                         o0 + c0:o0 + c0 + w],
                                in_=of[:])

            # ---- prologue ----
            x_tiles = {0: load_x(0)}
            nc.sync.dma_start(out=w_all[:, 0:12 * P],
                              in_=wts[:, 0:12 * P])
            x_tiles[1] = load_x(1)
            nc.sync.dma_start(out=w_all[:, 12 * P:48 * P],
                              in_=wts[:, 12 * P:48 * P])
            for q in range(2):
                lo = (48 + q * 24) * P
                hi = (48 + (q + 1) * 24) * P
                nc.sync.dma_start(out=w_all[:, lo:hi], in_=wts[:, lo:hi])
            nc.sync.dma_start(out=w_all[:, NW * P:(NW + 1) * P],
                              in_=wts[:, NW * P:(NW + 1) * P])
            z1s = {0: emit_act1(0, x_tiles[0]), 1: emit_act1(1, x_tiles[1])}
            z2cur = emit_conv1(0, z1s.pop(0))

            # ---- software pipeline: act chains lead their convs by a
            # full iteration so the PE never waits on a same-iteration
            # activation chain ----
            for c in range(len(CHUNKS)):
                if c + 2 <= last_i and (c + 2) not in x_tiles:
                    x_tiles[c + 2] = load_x(c + 2)
                z3 = emit_act2(c, z2cur)
                if c + 2 <= last_i:
                    z1s[c + 2] = emit_act1(c + 2, x_tiles[c + 2])
                if c + 1 <= last_i:
                    z2cur = emit_conv1(c + 1, z1s.pop(c + 1))
                emit_conv2(c, z3, x_tiles.pop(c),
                           evict_on_dve=(c >= last_i - 1))
    nc.compile()
    return nc


def _host_prep(x, v1, g1, bias1, v2, g2, bias2, alpha1, beta1, alpha2, beta2):
    f32 = np.float32

    def wn(v, g):
        nrm = np.sqrt((v * v).sum(axis=(1, 2), keepdims=True))
        return (g[:, None, None] * v / nrm).astype(f32)

    def bf(a):
        return np.asarray(a, dtype=f32).astype(ml_dtypes.bfloat16)

    a1 = np.exp(alpha1).astype(f32)
    a2 = np.exp(alpha2).astype(f32)
    rbp1 = ((4.0 / 3.0) / (2.0 * np.exp(beta1) + EPS)).astype(f32)
    rbp2 = ((4.0 / 3.0) / (2.0 * np.exp(beta2) + EPS)).astype(f32)
    P1 = (TWOPI / (1.5 * a1)).astype(f32)
    P2 = (TWOPI / (1.5 * a2)).astype(f32)
    d1 = ((3.0 / 16.0) * P1).astype(f32)
    d2 = ((3.0 / 16.0) * P2).astype(f32)

    W1 = wn(v1, g1) * f32(0.75)
    W2 = wn(v2, g2) * f32(0.75)

    NW = 2 * 3 * NCC * NCC
    wflat = np.zeros((P, (NW + 1) * P), dtype=ml_dtypes.bfloat16)
    wflat[:, NW * P:] = np.eye(P, dtype=f32).astype(ml_dtypes.bfloat16)
    Wq = {}
    for conv, W in ((0, W1), (1, W2)):
        Wb = bf(W)
        Wq[conv] = Wb.astype(f32)
        for dk in range(3):
            for ci in range(NCC):
                for co in range(NCC):
                    idx = conv * 48 + co * 12 + dk * NCC + ci
                    blk = Wb[co * P:(co + 1) * P, ci * P:(ci + 1) * P, dk]
                    wflat[:, idx * P:(idx + 1) * P] = blk.T

    sent1 = (P1 / 4 - rbp1).astype(f32)
    sent2 = (P2 / 4 - rbp2).astype(f32)

    beff1 = (bias1 + np.einsum('oik,i->o', Wq[0], rbp1 - P1 / 4) + d2
             ).astype(f32)
    beff2 = (bias2 + np.einsum('oik,i->o', Wq[1], rbp2 - P2 / 4) - d1
             ).astype(f32)

    prm_c = np.zeros((C, 16), dtype=f32)
    prm_c[:, 0] = 1.5 * a1 / TWOPI   # hh scale act1
    prm_c[:, 1] = 1.5 * a1           # cB scale on dz6
    prm_c[:, 2] = -rbp1              # gg scale (cA = +cos)
    prm_c[:, 3] = sent1
    prm_c[:, 5] = beff1
    prm_c[:, 6] = 1.5 * a2 / TWOPI
    prm_c[:, 7] = 0.25 * a2          # cB scale on raw dz
    prm_c[:, 8] = -rbp2
    prm_c[:, 9] = sent2
    prm_c[:, 11] = beff2
    prm = np.ascontiguousarray(
        prm_c.reshape(NCC, P, 16).transpose(1, 0, 2).reshape(P, NCC * 16))

    xpad = np.pad(x + d1[None, :, None], ((0, 0), (0, 0), (PAD, PAD)),
                  mode='edge').astype(f32)
    xpb = xpad.astype(ml_dtypes.bfloat16)
    x6b = (xpad / 6.0).astype(ml_dtypes.bfloat16)
    return xpb, x6b, wflat, prm


def kernel(x, v1, g1, bias1, v2, g2, bias2, alpha1, beta1, alpha2, beta2,
           _profile=False):
    x = np.ascontiguousarray(x, dtype=np.float32)
    xpb, x6b, wflat, prm = _host_prep(
        x, v1, g1, bias1, v2, g2, bias2, alpha1, beta1, alpha2, beta2)
    if 'nc' not in _NC_CACHE:
        _NC_CACHE['nc'] = build_nc()
    nc = _NC_CACHE['nc']
    B = x.shape[0]
    assert B == 8, f"expected B=8, got {B}"
    in_maps = [{"xpb": np.ascontiguousarray(xpb[b]),
                "x6b": np.ascontiguousarray(x6b[b]),
                "wts": wflat, "prm": prm} for b in range(B)]
    last_exc = None
    for attempt in range(3):
        try:
            res = run_bass_kernel_spmd(nc, in_maps, list(range(8)),
                                       trace=_profile)
            break
        except Exception as e:
            last_exc = e
            import time
            time.sleep(2.0)
    else:
        raise last_exc
    out = np.stack([res.results[b]["out"].astype(np.float32)
                    for b in range(B)])
    if _profile:
        kernel.last_results = res
    return out
